# revision 1
# baseline (speedup 1.0000x reference)
"""AGSM Trainium2 kernel: attention-gated temporal shift module on 8 NeuronCores.

Sharding: data-parallel over clips. B=16 clips; core k handles clips (2k, 2k+1)
= input rows [16k, 16k+16). BN batch stats cross-core via a tiny AllReduce.

Per-core layout: partitions 0:64 = clip A channels, 64:128 = clip B channels.
Spatial planes stored padded to 58x58 (zero borders) so conv taps are pure
free-axis offsets; X tensor has +-59 guard zeros so tap reads never leave it.
Convs run on TensorE as tap-accumulating matmuls with replicated-output
weights (attn/gates land pre-broadcast across the channel partitions).
"""
import numpy as np
import ml_dtypes

import concourse.bass as bass
import concourse.tile as tile
from concourse import mybir
from concourse.bass_utils import run_bass_kernel_spmd

N_CORES = 8
T = 8
PS = 58 * 58          # padded plane
GUARD = 59
XLEN = GUARD + T * PS + GUARD
BLEN = GUARD + PS + GUARD
QUAR = 841            # PS / 4
EPS = 1e-5
NTOT = 16 * T * 56 * 56
F32 = mybir.dt.float32
BF16 = mybir.dt.bfloat16
AF = mybir.ActivationFunctionType
ALU = mybir.AluOpType

_CACHE = {}


def _legalize_waits(nc):
    """This walrus accepts <=1 sync wait per instruction (2 for EventSemaphore).
    Hoist excess waits onto fresh same-engine NoOps inserted just before."""
    n = [0]
    for f in nc.m.functions:
        for bb in f.blocks:
            insts = bb.instructions  # live list
            i = 0
            while i < len(insts):
                inst = insts[i]
                si = inst.sync_info
                cap = 2 if type(inst).__name__ == "InstEventSemaphore" else 1
                if si is not None and len(si.on_wait) > cap:
                    waits = list(si.on_wait)
                    si.on_wait = waits[-cap:]
                    inst.sync_info = si
                    for w in waits[:-cap]:
                        n[0] += 1
                        nop = mybir.InstNoOp(
                            name=f"waitfix-{n[0]}", engine=inst.engine,
                            bass_nofuse=True,
                            sync_info=mybir.SyncInfo(on_wait=[w], on_update=[]))
                        nc.register_instruction(nop, overwrite=True)
                        insts.insert(i, nop)
                        i += 1
                i += 1


def _interior(ap_tensor, base_off):
    """2D interior AP [*, 56, 56] of a padded 58x58 plane at base_off."""
    return ap_tensor  # helper unused; slicing done inline


def build_nc(trace_sim=False, repeat=1):
    nc = bass.Bass(num_devices=N_CORES)
    x_ext = nc.declare_dram_parameter("x", [16, 64, 56, 56], F32, isOutput=False)
    w2d_e = nc.declare_dram_parameter("w2d", [128, 9 * 64], BF16, isOutput=False)
    w3d_e = nc.declare_dram_parameter("w3d", [128, 27 * 64], BF16, isOutput=False)
    b2_e = nc.declare_dram_parameter("b2", [128, 1], F32, isOutput=False)
    b3_e = nc.declare_dram_parameter("b3", [128, 1], F32, isOutput=False)
    gam_e = nc.declare_dram_parameter("gam", [128, 1], F32, isOutput=False)
    bet_e = nc.declare_dram_parameter("bet", [128, 1], F32, isOutput=False)
    out_ext = nc.declare_dram_parameter("out", [16, 64, 56, 56], F32, isOutput=True)

    with tile.TileContext(nc, trace_sim=trace_sim) as tc:
        with (
            tc.tile_pool(name="const", bufs=1) as cpool,
            tc.tile_pool(name="xbuf", bufs=1) as xpool,
            tc.tile_pool(name="attn", bufs=2) as apool,
            tc.tile_pool(name="bnr", bufs=4) as bpool,
            tc.tile_pool(name="gate", bufs=2) as gpool,
            tc.tile_pool(name="pbuf", bufs=4) as ppool,
            tc.tile_pool(name="ybuf", bufs=2) as ypool,
            tc.tile_pool(name="scr", bufs=1) as spool,
            tc.tile_pool(name="psum", bufs=2, space=bass.MemorySpace.PSUM) as psum,
            tc.tile_pool(name="dram", bufs=1, space="DRAM") as dram,
        ):
            # ---- constants ----
            w2d = cpool.tile([128, 9 * 64], BF16)
            w3d = cpool.tile([128, 27 * 64], BF16)
            b2 = cpool.tile([128, 1], F32)
            b3 = cpool.tile([128, 1], F32)
            gam = cpool.tile([128, 1], F32)
            bet = cpool.tile([128, 1], F32)
            nc.sync.dma_start(w2d[:], w2d_e[:])
            nc.sync.dma_start(w3d[:], w3d_e[:])
            nc.sync.dma_start(b2[:], b2_e[:])
            nc.sync.dma_start(b3[:], b3_e[:])
            nc.sync.dma_start(gam[:], gam_e[:])
            nc.sync.dma_start(bet[:], bet_e[:])

            X = xpool.tile([128, XLEN], BF16)
            Z = xpool.tile([128, BLEN], BF16)
            nc.vector.memset(X[:], 0.0)
            nc.vector.memset(Z[:], 0.0)
            sums = cpool.tile([128, T], F32)
            sumsqs = cpool.tile([128, T], F32)
            scratch = spool.tile([128, PS], BF16)

            def xpl(u, off=0, ln=PS):
                return X[:, GUARD + u * PS + off: GUARD + u * PS + off + ln]

            def xpl_h(rows, u, off, ln):
                return X[rows[0]:rows[1],
                         GUARD + u * PS + off: GUARD + u * PS + off + ln]

            # ---- input DMA (gpsimd SWDGE casts f32 -> bf16) ----
            body_state = {}
            def xint(rows, u):
                # [rows, 56, 56] interior of plane u (row stride 58)
                base = GUARD + u * PS + 59
                ap = X[rows[0]:rows[1], base: base + 56 * 58]
                return ap.rearrange("p (h w) -> p h w", h=56)[:, :, 0:56]

            for _rep in range(repeat):
                for u in range(T):
                    nc.gpsimd.dma_start(xint((0, 64), u), x_ext[u])
                    nc.gpsimd.dma_start(xint((64, 128), u), x_ext[8 + u])

                # ---- phase A: conv2d + sigmoid + gating + stats ----
                attn_tiles = {}
                for pair in range(4):
                    u0, u1 = 2 * pair, 2 * pair + 1
                    a0 = apool.tile([128, PS], BF16, name=f"attn{u0}", tag="attn")
                    a1 = apool.tile([128, PS], BF16, name=f"attn{u1}", tag="attn")
                    attn_tiles[u0], attn_tiles[u1] = a0, a1
                    for q in range(4):
                        p0 = psum.tile([128, QUAR], F32, name=f"a_ps0_{pair}_{q}", tag="ps0")
                        p1 = psum.tile([128, QUAR], F32, name=f"a_ps1_{pair}_{q}", tag="ps1")
                        for ck, ln in ((0, 512), (512, 329)):
                            for o in range(9):
                                soff = (o // 3 - 1) * 58 + (o % 3 - 1)
                                st, sp = o == 0, o == 8
                                base = q * QUAR + ck + soff
                                w = w2d[:, o * 64:(o + 1) * 64]
                                nc.tensor.matmul(p0[0:64, ck:ck + ln], w[0:64, :],
                                                 xpl_h((0, 64), u0, base, ln),
                                                 start=st, stop=sp)
                                nc.tensor.matmul(p0[64:128, ck:ck + ln], w[64:128, :],
                                                 xpl_h((64, 128), u0, base, ln),
                                                 start=st, stop=sp)
                                nc.tensor.matmul(p1[0:64, ck:ck + ln], w[64:128, :],
                                                 xpl_h((64, 128), u1, base, ln),
                                                 start=st, stop=sp)
                                nc.tensor.matmul(p1[64:128, ck:ck + ln], w[0:64, :],
                                                 xpl_h((0, 64), u1, base, ln),
                                                 start=st, stop=sp)
                        qs = slice(q * QUAR, (q + 1) * QUAR)
                        nc.scalar.activation(a0[:, qs], p0[:, :], AF.Sigmoid,
                                             bias=b2[:, 0:1])
                        nc.scalar.activation(a1[0:64, qs], p1[64:128, :], AF.Sigmoid,
                                             bias=b2[0:64, 0:1])
                        nc.scalar.activation(a1[64:128, qs], p1[0:64, :], AF.Sigmoid,
                                             bias=b2[0:64, 0:1])
                    for u, at in ((u0, a0), (u1, a1)):
                        nc.vector.scalar_tensor_tensor(
                            out=xpl(u), in0=at[:, :], scalar=0.5, in1=xpl(u),
                            op0=ALU.max, op1=ALU.mult,
                            accum_out=sums[:, u:u + 1])
                        nc.vector.scalar_tensor_tensor(
                            out=scratch[:, :], in0=xpl(u), scalar=0.0, in1=xpl(u),
                            op0=ALU.bypass, op1=ALU.mult,
                            accum_out=sumsqs[:, u:u + 1])

                # ---- BN stats: reduce, all-reduce, scale/bias ----
                red = cpool.tile([128, 2], F32)
                nc.vector.tensor_reduce(red[:, 0:1], sums[:], mybir.AxisListType.X,
                                        ALU.add)
                nc.vector.tensor_reduce(red[:, 1:2], sumsqs[:], mybir.AxisListType.X,
                                        ALU.add)
                cc_in = dram.tile([128, 2], F32)
                cc_out = dram.tile([128, 2], F32, addr_space="Shared")
                nc.sync.dma_start(cc_in[:], red[:])
                nc.gpsimd.collective_compute(
                    "AllReduce", ALU.add, replica_groups=[list(range(N_CORES))],
                    ins=[cc_in[:].opt()], outs=[cc_out[:].opt()])
                ar = cpool.tile([128, 2], F32)
                nc.sync.dma_start(ar[:], cc_out[:])

                st = cpool.tile([64, 8], F32)  # work area for stats math
                arl = cpool.tile([64, 2], F32)
                nc.vector.tensor_copy(arl[:, :], ar[64:128, :])
                # tot = A half + B half
                nc.vector.tensor_tensor(st[:, 0:2], ar[0:64, :], arl[:, :],
                                        ALU.add)
                # mean (col 0), E[x^2] (col 1)
                nc.vector.tensor_scalar_mul(st[:, 2:4], st[:, 0:2], 1.0 / NTOT)
                # var = E[x^2] - mean^2 (col 4); then + eps
                nc.vector.tensor_tensor(st[:, 4:5], st[:, 2:3], st[:, 2:3], ALU.mult)
                nc.vector.tensor_tensor(st[:, 4:5], st[:, 3:4], st[:, 4:5],
                                        ALU.subtract)
                nc.vector.tensor_scalar_add(st[:, 4:5], st[:, 4:5], EPS)
                # rsqrt: s = sqrt(v); r = 1/s (HW divide); Newton: r = r*(2 - s*r)
                nc.scalar.activation(st[:, 5:6], st[:, 4:5], AF.Sqrt)
                nc.vector.reciprocal(st[:, 6:7], st[:, 5:6])
                nc.vector.tensor_tensor(st[:, 7:8], st[:, 5:6], st[:, 6:7], ALU.mult)
                nc.vector.tensor_scalar(st[:, 7:8], st[:, 7:8], -1.0, 2.0,
                                        ALU.mult, ALU.add)
                nc.vector.tensor_tensor(st[:, 6:7], st[:, 6:7], st[:, 7:8], ALU.mult)
                scb = cpool.tile([128, 2], F32)  # col0 = scale, col1 = bias
                nc.vector.tensor_tensor(scb[0:64, 0:1], gam[0:64, :], st[:, 6:7],
                                        ALU.mult)
                nc.vector.tensor_tensor(st[:, 7:8], st[:, 2:3], scb[0:64, 0:1],
                                        ALU.mult)
                nc.vector.tensor_tensor(scb[0:64, 1:2], bet[0:64, :], st[:, 7:8],
                                        ALU.subtract)
                nc.vector.tensor_copy(scb[64:128, :], scb[0:64, :])

                # ---- phase B: bnrelu + conv3d + tanh + combine + out ----
                def bint(tile_, rows, q=None):
                    base = GUARD + 59
                    ap = tile_[rows[0]:rows[1], base: base + 56 * 58]
                    return ap.reshape([rows[1] - rows[0], 56, 58])[:, :, 0:56]

                bnr_tiles = {}

                def make_bnr(u):
                    t = bpool.tile([128, BLEN], BF16, name=f"bnr{u}", tag="bnr")
                    if u < 4:
                        nc.vector.memset(t[:], 0.0)
                    base = GUARD + u * PS + 59
                    src = X[:, base: base + 56 * 58].rearrange(
                        "p (h w) -> p h w", h=56)[:, :, 0:56]
                    dst = t[:, GUARD + 59: GUARD + 59 + 56 * 58].rearrange(
                        "p (h w) -> p h w", h=56)[:, :, 0:56]
                    nc.scalar.activation(dst, src, AF.Relu,
                                         bias=scb[:, 1:2], scale=scb[:, 0:1])
                    bnr_tiles[u] = t

                P_tiles = {}
                y_count = [0]

                def emit_out(u, y):
                    yim = y[:, 59:59 + 56 * 58].rearrange(
                        "p (h w) -> p h w", h=56)[:, :, 0:56]
                    for (p0, smp, ch0) in ((0, u, 0), (32, u, 32),
                                           (64, 8 + u, 0), (96, 8 + u, 32)):
                        for j in range(2):
                            src = yim[p0 + 16 * j: p0 + 16 * (j + 1)]
                            dst = out_ext[smp, ch0 + j: ch0 + 32: 2, :, :]
                            nc.sync.dma_start(dst, src)

                def combine(u):
                    y = ypool.tile([128, PS], F32, name=f"y{u}", tag="y")
                    nc.vector.tensor_tensor(y[:], xpl(u), P_tiles[u][:, :],
                                            ALU.subtract)
                    for (r0, r1, du, eng) in ((0, 32, 1, nc.vector),
                                              (32, 64, -1, nc.vector),
                                              (64, 96, 1, nc.gpsimd),
                                              (96, 128, -1, nc.gpsimd)):
                        un = u + du
                        if 0 <= un < T:
                            eng.tensor_tensor(
                                y[r0:r1], y[r0:r1], P_tiles[un][r0:r1, :], ALU.add)
                    emit_out(u, y)

                make_bnr(0)
                for pair in range(4):
                    u0, u1 = 2 * pair, 2 * pair + 1
                    if u1 < T:
                        make_bnr(u1)
                    if u1 + 1 < T:
                        make_bnr(u1 + 1)
                    g0 = gpool.tile([128, PS], BF16, name=f"gate{u0}", tag="gate")
                    g1 = gpool.tile([128, PS], BF16, name=f"gate{u1}", tag="gate")
                    for q in range(4):
                        p0 = psum.tile([128, QUAR], F32, name=f"g_ps0_{pair}_{q}", tag="ps0")
                        p1 = psum.tile([128, QUAR], F32, name=f"g_ps1_{pair}_{q}", tag="ps1")
                        for ck, ln in ((0, 512), (512, 329)):
                            for o in range(27):
                                bdt = o // 9
                                soff = ((o // 3) % 3 - 1) * 58 + (o % 3 - 1)
                                st_, sp_ = o == 0, o == 26
                                base = GUARD + q * QUAR + ck + soff
                                w = w3d[:, o * 64:(o + 1) * 64]
                                us0, us1 = u0 + bdt - 1, u1 + bdt - 1
                                m0 = (bnr_tiles[us0] if 0 <= us0 < T else Z)
                                m1 = (bnr_tiles[us1] if 0 <= us1 < T else Z)
                                nc.tensor.matmul(p0[0:64, ck:ck + ln], w[0:64, :],
                                                 m0[0:64, base:base + ln],
                                                 start=st_, stop=sp_)
                                nc.tensor.matmul(p0[64:128, ck:ck + ln], w[64:128, :],
                                                 m0[64:128, base:base + ln],
                                                 start=st_, stop=sp_)
                                nc.tensor.matmul(p1[0:64, ck:ck + ln], w[64:128, :],
                                                 m1[64:128, base:base + ln],
                                                 start=st_, stop=sp_)
                                nc.tensor.matmul(p1[64:128, ck:ck + ln], w[0:64, :],
                                                 m1[0:64, base:base + ln],
                                                 start=st_, stop=sp_)
                        qs = slice(q * QUAR, (q + 1) * QUAR)
                        nc.scalar.activation(g0[:, qs], p0[:, :], AF.Tanh,
                                             bias=b3[:, 0:1])
                        nc.scalar.activation(g1[0:64, qs], p1[64:128, :], AF.Tanh,
                                             bias=b3[0:64, 0:1])
                        nc.scalar.activation(g1[64:128, qs], p1[0:64, :], AF.Tanh,
                                             bias=b3[0:64, 0:1])
                    for u, g in ((u0, g0), (u1, g1)):
                        pt = ppool.tile([128, PS], BF16, name=f"P{u}", tag="pt")
                        nc.vector.tensor_tensor(pt[:], g[:, :], xpl(u), ALU.mult)
                        P_tiles[u] = pt
                    # combines for which P[u+1] now exists
                    if pair > 0:
                        combine(u0 - 1)
                    combine(u0)
                    if pair == 3:
                        combine(u1)

    return nc


def _host_prep(attn_w, attn_b, bn_gamma, bn_beta, conv3d_w, conv3d_b):
    W2 = np.zeros((128, 9 * 64), np.float32)
    for o in range(9):
        dy, dx = o // 3, o % 3
        for c in range(64):
            W2[0:64, o * 64 + c] = attn_w[c // 16, :, dy, dx]
    W2[64:128] = W2[0:64]
    W3 = np.zeros((128, 27 * 64), np.float32)
    for o in range(27):
        dt, dy, dx = o // 9, (o // 3) % 3, o % 3
        W3[0:32, o * 64 + 0:o * 64 + 32] = conv3d_w[0, :, dt, dy, dx][:, None]
        W3[32:64, o * 64 + 32:o * 64 + 64] = conv3d_w[1, :, dt, dy, dx][:, None]
    W3[64:128] = W3[0:64]
    b2r = np.zeros((128, 1), np.float32)
    b3r = np.zeros((128, 1), np.float32)
    gr = np.zeros((128, 1), np.float32)
    br = np.zeros((128, 1), np.float32)
    for c in range(64):
        b2r[c, 0] = b2r[c + 64, 0] = attn_b[c // 16]
        b3r[c, 0] = b3r[c + 64, 0] = conv3d_b[c // 32]
        gr[c, 0] = gr[c + 64, 0] = bn_gamma[c]
        br[c, 0] = br[c + 64, 0] = bn_beta[c]
    bf = ml_dtypes.bfloat16
    return (W2.astype(bf), W3.astype(bf), b2r, b3r, gr, br)


def get_nc():
    if "nc" not in _CACHE:
        nc = build_nc()
        _legalize_waits(nc)
        _CACHE["nc"] = nc
    return _CACHE["nc"]


def make_in_maps(x, attn_w, attn_b, bn_gamma, bn_beta, conv3d_w, conv3d_b):
    W2, W3, b2r, b3r, gr, br = _host_prep(
        np.asarray(attn_w), np.asarray(attn_b), np.asarray(bn_gamma),
        np.asarray(bn_beta), np.asarray(conv3d_w), np.asarray(conv3d_b))
    x = np.asarray(x)
    in_maps = []
    for k in range(N_CORES):
        in_maps.append({
            "x": np.ascontiguousarray(x[16 * k:16 * (k + 1)]),
            "w2d": W2, "w3d": W3, "b2": b2r, "b3": b3r,
            "gam": gr, "bet": br,
        })
    return in_maps


def kernel(x, attn_w, attn_b, bn_gamma, bn_beta, conv3d_w, conv3d_b):
    nc = get_nc()
    in_maps = make_in_maps(x, attn_w, attn_b, bn_gamma, bn_beta,
                           conv3d_w, conv3d_b)
    res = run_bass_kernel_spmd(nc, in_maps, core_ids=list(range(N_CORES)))
    out = np.concatenate([res.results[k]["out"] for k in range(N_CORES)], 0)
    return out.astype(np.float32)

